# revision 26
# baseline (speedup 1.0000x reference)
"""Trainium2 Bass kernel for DenseDilatedKnnGraph (B=4, C=192, N=M=3136, K=9).

Computes, per batch: L2-normalize x,y over channels; dist = cdist(xn, yn) +
relative_pos; output the indices of the 9 smallest distances per query row,
stacked with the center indices -> (2, B, N, 9) int32.

Sharding: one (batch, half-N) block per core: core i handles batch i//2,
query rows (i%2)*1568..(i%2+1)*1568.  Each core normalizes only its own
batch's y, and processes 13 row-tiles of 128 rows (12x128 + 32).

Per-core pipeline (row-tile t of 128 rows):
  PE:   psum = x^T yn via TWO f32r matmuls per 512-col chunk (full fp32
        accuracy at bf16-class speed for moving dim >= 256)
  ACT:  s = sqrt(psum * (-2/||x_row||) + 2)      [d^2 = 2 - 2 cos]
  Pool: neg = (-relpos) - s                      [negated distance; relpos is
        negated host-side so this is one TensorTensor subtract -- the Pool
        engine has no TensorScalarPtr opcode]
  DVE:  segmented top-k: 4x (max8 + find_index8) over 784-wide segments
        -> 32 (value, index) candidates; small merge (max8/match_replace/
        max8 on 32) gives the global top-9 values; arithmetic index gather:
        for each of the 9 values, one scalar_tensor_tensor (is_equal, mult,
        accum) over the 32 candidates yields its global column index
"""

import numpy as np

import concourse.bacc as bacc
import concourse.bass as bass
import concourse.mybir as mybir
import concourse.tile as tile
from concourse.bass_utils import run_bass_kernel_spmd

B, C, N, M, K = 4, 192, 3136, 3136, 9
NCORES = 8
HALF = N // 2                 # 1568 query rows per core
TR = 128                      # rows per compute tile
NT = (HALF + TR - 1) // TR    # 13 tiles (12 full + one of 32 rows)
C0, C1 = 128, 64              # contraction split of C=192
NSEG = 4                      # top-k segments per row
W = M // NSEG                 # 784
NC_ = NSEG * 8                # 32 merge candidates

# psum column chunks: bank-aligned (512 fp32 = one 2KB bank), 3136 = 6*512+64
CHUNKS = [(i * 512, min(512, M - i * 512)) for i in range((M + 511) // 512)]

F32 = mybir.dt.float32
F32R = mybir.dt.float32r
U16 = mybir.dt.uint16
U32 = mybir.dt.uint32
NEG_BIG = -3.0e38
Alu = mybir.AluOpType
Act = mybir.ActivationFunctionType


def _rows(t):
    return TR if t < NT - 1 else HALF - TR * (NT - 1)


def _build_kernel():
    nc = bacc.Bacc("TRN2", target_bir_lowering=False, debug=False,
                   num_devices=NCORES)
    # x is only consumed by f32r matmuls / ACT square; declare f32r so the
    # DMA lands directly in matmul-ready dtype (bits are plain fp32).
    x_ap = nc.dram_tensor("x_blk", [C, HALF], F32R, kind="ExternalInput").ap()
    y_ap = nc.dram_tensor("y_full", [C, M], F32, kind="ExternalInput").ap()
    rp_ap = nc.dram_tensor("relpos", [HALF, M], F32, kind="ExternalInput").ap()
    out_ap = nc.dram_tensor("out_idx", [HALF, K], U32,
                            kind="ExternalOutput").ap()

    with tile.TileContext(nc) as tc:
        _emit(tc, out_ap, x_ap, y_ap, rp_ap)
    nc.compile()
    return nc


def _emit(tc, out_ap, x_ap, y_ap, rp_ap):
    nc = tc.nc
    from contextlib import ExitStack
    with ExitStack() as ctx:
        const_p = ctx.enter_context(tc.tile_pool(name="const", bufs=1))
        x_p = ctx.enter_context(tc.tile_pool(name="x", bufs=1))
        y_p = ctx.enter_context(tc.tile_pool(name="y", bufs=1))
        big_p = ctx.enter_context(tc.tile_pool(name="big", bufs=1))
        rp_p = ctx.enter_context(tc.tile_pool(name="rp", bufs=3))
        s_p = ctx.enter_context(tc.tile_pool(name="s", bufs=3))
        neg_p = ctx.enter_context(tc.tile_pool(name="neg", bufs=3))
        sm_p = ctx.enter_context(tc.tile_pool(name="sm", bufs=2))
        ps_p = ctx.enter_context(tc.tile_pool(name="ps", bufs=4, space="PSUM"))
        psy_p = ctx.enter_context(tc.tile_pool(name="psy", bufs=2,
                                               space="PSUM"))
        psn_p = ctx.enter_context(tc.tile_pool(name="psn", bufs=1,
                                               space="PSUM"))

        # ---- constants ----
        onesf = const_p.tile([128, 128], F32, tag="onesf")
        nc.vector.memset(onesf[:, :], 1.0)
        ones = const_p.tile([128, 128], F32R, tag="ones")
        nc.vector.tensor_copy(ones[:, :], onesf[:, :])
        two_col = const_p.tile([TR, 1], F32, tag="two")
        nc.vector.memset(two_col[:, :], 2.0)
        # global column offset of each candidate slot (8 per segment)
        offs = const_p.tile([TR, NC_], F32, tag="offs")
        for s in range(NSEG):
            nc.vector.memset(offs[:, 8 * s:8 * (s + 1)], float(W * s))

        # ---- x load (first: it gates the scale chain) + squares ----
        x0 = x_p.tile([C0, HALF], F32R, tag="x0")
        x1 = x_p.tile([C1, HALF], F32R, tag="x1")
        sqx0 = big_p.tile([C0, HALF], F32R, tag="bigA")
        sqx1 = big_p.tile([C1, HALF], F32R, tag="bigB")
        XCH = [(i * 512, min(512, HALF - i * 512))
               for i in range((HALF + 511) // 512)]
        for lo_c, sz in XCH:
            cs = slice(lo_c, lo_c + sz)
            nc.sync.dma_start(x0[:, cs], x_ap[0:C0, cs])
            nc.sync.dma_start(x1[:, cs], x_ap[C0:C, cs])
            nc.scalar.activation(sqx0[:, cs], x0[:, cs], Act.Square)
            nc.scalar.activation(sqx1[:, cs], x1[:, cs], Act.Square)
        # row norms via free=2 f32r matmuls (even free: fp32r-ISA legal)
        # straight into a [row, 2*tile] psum layout -- no transposes/DMAs.
        nx2 = psn_p.tile([TR, 2 * NT], F32, tag="nx2")
        nc.vector.memset(nx2[:, :], 1.0)
        for t in range(NT):
            rt = _rows(t)
            lo = t * TR
            nc.tensor.matmul(nx2[0:rt, 2 * t:2 * t + 2], sqx0[:, lo:lo + rt],
                             ones[0:C0, 0:2], start=True, stop=False)
            nc.tensor.matmul(nx2[0:rt, 2 * t:2 * t + 2], sqx1[:, lo:lo + rt],
                             ones[0:C1, 0:2], start=False, stop=True)
        nxs = const_p.tile([TR, 2 * NT], F32, tag="nxs")
        nc.scalar.activation(nxs[:, :], nx2[:, :], Act.Sqrt)
        nxr = const_p.tile([TR, 2 * NT], F32, tag="nxr")
        nc.vector.reciprocal(nxr[:, :], nxs[:, :])
        scale = const_p.tile([TR, 2 * NT], F32, tag="scale")
        nc.vector.tensor_scalar_mul(scale[:, :], nxr[:, :], -2.0)

        # ---- y load + normalize (chunk-pipelined) ----
        # y0/y1/ny are dead after this phase; they are tag-chained into the
        # s/neg rotation slots so those pools get 3-deep buffering for free.
        y0 = neg_p.tile([C0, M], F32, tag="neg", name="y0")
        y1 = s_p.tile([C1, M], F32, tag="s", name="y1")
        sq0 = big_p.tile([C0, M], F32R, tag="bigA")
        sq1 = big_p.tile([C1, M], F32R, tag="bigB")
        ny = s_p.tile([C0, M], F32, tag="s")        # chained into s slots
        nyr = y_p.tile([C0, M], F32, tag="nyr")
        yr0 = y_p.tile([C0, M], F32R, tag="yr0")
        yr1 = y_p.tile([C1, M], F32R, tag="yr1")
        for lo_c, sz in CHUNKS:
            cs = slice(lo_c, lo_c + sz)
            nc.sync.dma_start(y0[:, cs], y_ap[0:C0, cs])
            nc.sync.dma_start(y1[:, cs], y_ap[C0:C, cs])
            nc.scalar.activation(sq0[:, cs], y0[:, cs], Act.Square)
            nc.scalar.activation(sq1[:, cs], y1[:, cs], Act.Square)
            ny2 = psy_p.tile([C0, 512], F32, tag="ny2")
            nc.tensor.matmul(ny2[:, 0:sz], ones[0:C0, :], sq0[:, cs],
                             start=True, stop=False)
            nc.tensor.matmul(ny2[:, 0:sz], ones[0:C1, 0:128], sq1[:, cs],
                             start=False, stop=True)
            nc.scalar.activation(ny[:, cs], ny2[:, 0:sz], Act.Sqrt)
            nc.vector.reciprocal_approx_fast(nyr[:, cs], ny[:, cs])
            nc.gpsimd.tensor_tensor(yr0[:, cs], y0[:, cs], nyr[0:C0, cs],
                                    op=Alu.mult)
            nc.gpsimd.tensor_tensor(yr1[:, cs], y1[:, cs], nyr[0:C1, cs],
                                    op=Alu.mult)

        # ---- main row-tiles, software-pipelined ----
        rp_tiles = {}

        def load_rp(t):
            if t >= NT:
                return
            rt_ = _rows(t)
            rp_t = rp_p.tile([TR, M], F32, tag="rp", name=f"rp{t}")
            nc.sync.dma_start(rp_t[0:rt_, :], rp_ap[t * TR:t * TR + rt_, :])
            rp_tiles[t] = rp_t

        load_rp(0)
        load_rp(1)

        for t in range(NT):
            rt = _rows(t)
            lo = t * TR
            load_rp(t + 2)

            s_t = s_p.tile([TR, M], F32, tag="s", name=f"s{t}")
            # relpos arrives negated, so neg = (-rp) - s, one Pool subtract
            # per chunk: downstream DVE segments can start before the whole
            # row is done (subtile deps).
            neg = neg_p.tile([TR, M], F32, tag="neg", name=f"neg{t}")
            for lo_c, sz in CHUNKS:
                cs = slice(lo_c, lo_c + sz)
                pd = ps_p.tile([TR, 512], F32, tag="pd")
                nc.tensor.matmul(pd[0:rt, 0:sz], x0[:, lo:lo + rt],
                                 yr0[:, cs], start=True, stop=False)
                nc.tensor.matmul(pd[0:rt, 0:sz], x1[:, lo:lo + rt],
                                 yr1[:, cs], start=False, stop=True)
                nc.scalar.activation(s_t[0:rt, cs], pd[0:rt, 0:sz], Act.Sqrt,
                                     bias=two_col[0:rt, :],
                                     scale=scale[0:rt, 2 * t:2 * t + 1])
                nc.gpsimd.tensor_tensor(neg[0:rt, cs], rp_tiles[t][0:rt, cs],
                                        s_t[0:rt, cs], op=Alu.subtract)
            del rp_tiles[t]

            # segmented top-8 values + indices -> 32 candidates
            cv = sm_p.tile([TR, NC_], F32, tag="cv", name=f"cv{t}")
            ci = sm_p.tile([TR, NC_], U16, tag="ci", name=f"ci{t}")
            for sg in range(NSEG):
                ss = slice(sg * W, (sg + 1) * W)
                c8 = slice(sg * 8, sg * 8 + 8)
                nc.vector.max(out=cv[0:rt, c8], in_=neg[0:rt, ss])
                nc.vector.max_index(out=ci[0:rt, c8], in_max=cv[0:rt, c8],
                                    in_values=neg[0:rt, ss])
            # merge: global top-8 into v9a[:, 0:8], then the 9th
            v9a = sm_p.tile([TR, K], F32, tag="v9a", name=f"v9a{t}")
            nc.vector.max(out=v9a[0:rt, 0:8], in_=cv[0:rt, :])
            cvr = sm_p.tile([TR, NC_], F32, tag="cvr", name=f"cvr{t}")
            nc.vector.match_replace(out=cvr[0:rt, :],
                                    in_to_replace=v9a[0:rt, 0:8],
                                    in_values=cv[0:rt, :], imm_value=NEG_BIG)
            v9 = sm_p.tile([TR, 8], F32, tag="v9", name=f"v9{t}")
            nc.vector.max(out=v9[0:rt, :], in_=cvr[0:rt, :])
            nc.vector.tensor_copy(v9a[0:rt, 8:9], v9[0:rt, 0:1])

            # arithmetic index gather, batched over all 9 outputs:
            # idx[k] = sum_p (cv[p] == v9a[k]) * gif[p]
            cif = sm_p.tile([TR, NC_], F32, tag="cif", name=f"cif{t}")
            gif = sm_p.tile([TR, NC_], F32, tag="gif", name=f"gif{t}")
            nc.vector.tensor_copy(cif[0:rt, :], ci[0:rt, :])
            nc.vector.tensor_tensor(gif[0:rt, :], cif[0:rt, :],
                                    offs[0:rt, :], op=Alu.add)
            eq = sm_p.tile([TR, K, NC_], F32, tag="eq", name=f"eq{t}")
            v9b = v9a[0:rt, :].unsqueeze(2).broadcast_to((rt, K, NC_))
            cvb = cv[0:rt, :].unsqueeze(1).broadcast_to((rt, K, NC_))
            gfb = gif[0:rt, :].unsqueeze(1).broadcast_to((rt, K, NC_))
            nc.vector.tensor_tensor(eq[0:rt, :, :], v9b, cvb, op=Alu.is_equal)
            nc.vector.tensor_tensor(eq[0:rt, :, :], eq[0:rt, :, :], gfb,
                                    op=Alu.mult)
            idxf = sm_p.tile([TR, K], F32, tag="idxf", name=f"idxf{t}")
            nc.vector.tensor_reduce(idxf[0:rt, :], eq[0:rt, :, :],
                                    axis=mybir.AxisListType.X, op=Alu.add)
            idxu = sm_p.tile([TR, K], U32, tag="idxu", name=f"idxu{t}")
            nc.vector.tensor_copy(idxu[0:rt, :], idxf[0:rt, :])
            nc.sync.dma_start(out_ap[lo:lo + rt, :], idxu[0:rt, :])


_NC = None


def _get_nc():
    global _NC
    if _NC is None:
        _NC = _build_kernel()
    return _NC


def _run(inputs, trace=False, trace_kwargs=None):
    x = np.asarray(inputs["x"], dtype=np.float32)
    y = np.asarray(inputs["y"], dtype=np.float32)
    rp = np.asarray(inputs["relative_pos"], dtype=np.float32)
    assert x.shape == (B, C, N, 1) and y.shape == (B, C, M, 1)
    assert rp.shape == (1, N, M)

    in_maps = []
    for i in range(NCORES):
        b, h = divmod(i, 2)
        rs = slice(h * HALF, (h + 1) * HALF)
        in_maps.append({
            "x_blk": np.ascontiguousarray(x[b, :, rs, 0]),
            "y_full": np.ascontiguousarray(y[b, :, :, 0]),
            "relpos": np.ascontiguousarray(-rp[0, rs, :]),
        })
    nc = _get_nc()
    kwargs = {}
    if trace:
        kwargs = dict(trace=True, trace_cores=list(range(NCORES)),
                      trace_kwargs=trace_kwargs or {})
    res = run_bass_kernel_spmd(nc, in_maps, core_ids=list(range(NCORES)),
                               **kwargs)
    nn = np.empty((B, N, K), dtype=np.int32)
    for i in range(NCORES):
        b, h = divmod(i, 2)
        rs = slice(h * HALF, (h + 1) * HALF)
        nn[b, rs, :] = res.results[i]["out_idx"].view(np.int32)
    center = np.broadcast_to(np.arange(N, dtype=np.int32)[None, :, None],
                             (B, N, K))
    out = np.stack((nn, center), axis=0)
    return out, res


def kernel(**inputs):
    out, _ = _run(inputs, trace=False)
    return out
